# revision 2
# baseline (speedup 1.0000x reference)
"""DiffVG-style circle renderer on 8 Trainium2 NeuronCores.

Strategy: shard the 1024x1024 image by rows (128 rows per core). Each core
composites the circles whose vertical span intersects its row band,
front-to-back with transmittance T:

    w   = cov * T                      (w ring, fp16; cov = sigmoid(2(r-d)))
    T  += (-a) * w                     (transmittance chain)
    C_ch += (a*col_ch) * w             (via premultiplied m_ch ring)

Front-to-back order is relaxed: circles whose column windows don't overlap
commute, so each core emits a width-descending order compatible with the
z partial order. Slot k's window width is the max over cores of the k-th
emitted circle width (compile-time constant); offsets are runtime data.

v2 engine split: the serial T-chain lives ENTIRELY on DVE (cross-engine
dependent ping-pong costs ~550ns/hop on TRN2, measured):
  PE     z = (r^2 - d^2)/r outer-sum; two circles per K=8 matmul
         (bf16 hi/lo split operands), bias folded in -> no sqrt pass
  ACT    per-pair sigmoid PSUM -> fp16 cov ring; share of m premults
  Pool   share of m premults (tensor_scalar, 1-input ~1cyc/elem)
  DVE    w = cov*T (TT 2x), T += (-a)*w (stt, chain), share of m
         premults (ts 4x), single 3-plane batched C add (delayed 2 slots)
Premult channels are assigned to engines by a greedy balance over the
compile-time slot-width profile. State T and [CR|CG|CB] are fp16 planes;
output = 4 fp16 planes DMA'd out, assembled/converted to f32 on host.
"""

import sys

if "/opt/trn_rl_repo" not in sys.path:
    sys.path.insert(0, "/opt/trn_rl_repo")

import numpy as np
import ml_dtypes

import concourse.bass as bass
import concourse.bacc as bacc
import concourse.mybir as mybir
from concourse.tile import TileContext
from concourse import bass_utils

H = 1024
W = 1024
ROWS = 128
N_CORES = 8
MARGIN = 5.0
ROUND = 8
WMIN = 24
WCAP = 224
F32 = mybir.dt.float32
F16 = mybir.dt.float16
BF16 = mybir.dt.bfloat16
I32 = mybir.dt.int32
AF = mybir.ActivationFunctionType
OP = mybir.AluOpType
BF = ml_dtypes.bfloat16


# ---------------------------------------------------------------- host plan
def _core_circles(centers, radii, core):
    """Kept circle indices + cap-clipped rounded widths + offsets."""
    y0 = ROWS * core
    cy = centers[:, 1].astype(np.float64)
    cx = centers[:, 0].astype(np.float64)
    r = radii.astype(np.float64)
    keep = (cy + r + MARGIN >= y0 + 0.5) & (cy - r - MARGIN <= y0 + ROWS - 0.5)
    idx = np.where(keep)[0]
    dymin = np.maximum(0.0, np.maximum(y0 + 0.5 - cy[idx],
                                       cy[idx] - (y0 + ROWS - 0.5)))
    rm = r[idx] + MARGIN
    halfw = np.sqrt(np.maximum(rm * rm - dymin * dymin, 4.0))
    ws = np.clip(np.ceil(2.0 * halfw / ROUND) * ROUND, WMIN, WCAP).astype(int)
    off = np.clip(np.round(cx[idx] - ws / 2.0), 0, W - ws).astype(int)
    return idx, ws, off


def _greedy_f2b(idx, ws, off):
    """Front-to-back (topmost first) order, widest-available-first among
    circles whose later-drawn column-overlapping circles are all emitted."""
    n = len(idx)
    lo, hi = off, off + ws
    done = np.zeros(n, bool)
    order = []
    for _ in range(n):
        best, bestw = -1, -1
        for j in range(n):
            if done[j]:
                continue
            ok = True
            for p in range(n):
                if p == j or done[p]:
                    continue
                if idx[p] > idx[j] and lo[p] < hi[j] and lo[j] < hi[p]:
                    ok = False
                    break
            if ok and ws[j] > bestw:
                bestw, best = ws[j], j
        order.append(best)
        done[best] = True
    return np.array(order, int)


def make_plan(centers, radii):
    """Per-core ordered circle lists + global slot width profile."""
    percore = []
    for core in range(N_CORES):
        idx, ws, off = _core_circles(centers, radii, core)
        o = _greedy_f2b(idx, ws, off)
        percore.append((idx[o], ws[o], off[o]))
    S = max(len(p[0]) for p in percore)
    S = ((S + 1) // 2) * 2

    # swap-pass: adjacent column-disjoint circles commute; swap where it
    # lowers the per-slot cross-core max-width profile
    P = np.zeros((N_CORES, S), int)
    for c, (ids, ws, off) in enumerate(percore):
        P[c, :len(ws)] = ws
    for _ in range(8):
        changed = False
        for c in range(N_CORES):
            ids, ws, off = percore[c]
            for k in range(len(ws) - 1):
                if not (off[k] + ws[k] <= off[k + 1]
                        or off[k + 1] + ws[k + 1] <= off[k]):
                    continue
                others = np.delete(P, c, axis=0)
                ok = max(int(others[:, k].max()), WMIN)
                ok1 = max(int(others[:, k + 1].max()), WMIN)
                cur = max(ok, ws[k]) + max(ok1, ws[k + 1])
                new = max(ok, ws[k + 1]) + max(ok1, ws[k])
                if new < cur:
                    ids[k], ids[k + 1] = ids[k + 1], ids[k]
                    ws[k], ws[k + 1] = ws[k + 1], ws[k]
                    off[k], off[k + 1] = off[k + 1], off[k]
                    P[c, k], P[c, k + 1] = ws[k], ws[k + 1]
                    changed = True
        if not changed:
            break

    slotw = np.full(S, WMIN, int)
    for idx, ws, off in percore:
        slotw[:len(ws)] = np.maximum(slotw[:len(ws)], ws)
    return percore, slotw


def _hilo(x):
    hi = x.astype(BF)
    lo = (x - hi.astype(np.float64)).astype(BF)
    return hi, lo


def make_inputs(centers, radii, colors, plan):
    percore, slotw = plan
    S = len(slotw)
    npairs = S // 2
    pairw = [int(slotw[2 * i] + slotw[2 * i + 1]) for i in range(npairs)]
    assert all(pw <= 448 for pw in pairw)
    rhs_len = sum(pairw)
    pair_start = np.concatenate([[0], np.cumsum(pairw)]).astype(int)

    cy = centers[:, 1].astype(np.float64)
    cx = centers[:, 0].astype(np.float64)
    r = radii.astype(np.float64)
    col = colors.astype(np.float64)

    ins = []
    for core in range(N_CORES):
        y0 = ROWS * core
        ids, ws, offs_c = percore[core]
        n = len(ids)
        scal = np.zeros((ROWS, S * 4), np.float32)
        offs = np.zeros((1, S), np.int32)
        lhsT = np.zeros((8, npairs * ROWS), BF)
        rhs = np.zeros((8, rhs_len), BF)
        p = y0 + np.arange(ROWS, dtype=np.float64) + 0.5
        for k in range(n):
            i = ids[k]
            vk = int(slotw[k])
            off = int(np.clip(offs_c[k] + (ws[k] - vk) // 2, 0, W - vk))
            offs[0, k] = off
            al = col[i, 3]
            scal[:, k * 4 + 0] = -al
            scal[:, k * 4 + 1] = al * col[i, 0]
            scal[:, k * 4 + 2] = al * col[i, 1]
            scal[:, k * 4 + 3] = al * col[i, 2]
            j = off + np.arange(vk, dtype=np.float64) + 0.5
            a = r[i] / 2.0 - (p - cy[i]) ** 2 / r[i]
            b = r[i] / 2.0 - (j - cx[i]) ** 2 / r[i]
            ah, alo = _hilo(a)
            bh, blo = _hilo(b)
            pair, half = divmod(k, 2)
            rb = 4 * half
            ls = slice(pair * ROWS, (pair + 1) * ROWS)
            lhsT[rb + 0, ls] = ah
            lhsT[rb + 1, ls] = alo
            lhsT[rb + 2, ls] = 1.0
            lhsT[rb + 3, ls] = 1.0
            c0 = pair_start[pair] + (0 if half == 0 else int(slotw[2 * pair]))
            rs = slice(c0, c0 + vk)
            rhs[rb + 0, rs] = 1.0
            rhs[rb + 1, rs] = 1.0
            rhs[rb + 2, rs] = bh
            rhs[rb + 3, rs] = blo
        ins.append({"scal": scal, "offs": offs, "lhsT": lhsT, "rhs": rhs})
    return ins


# ------------------------------------------------- premult engine balancer
def _assign_premults(slotw):
    """Greedily assign each slot's 3 premult channels to ACT/Pool/DVE so the
    projected per-engine busy stays balanced. Costs in ns from microbench."""
    S = len(slotw)
    # fixed per-engine loads (ns)
    dve = 0.0
    act = 0.0
    pool = 0.0
    for k in range(S):
        vk = float(slotw[k])
        # DVE mandatory: TT w' + stt T + 3-plane C add
        dve += (58 + vk / 2) / 0.96 + 78        # TT w'
        dve += (58 + vk) / 0.96 + 60            # stt (AP scalar)
        dve += (58 + 3 * vk / 2) / 0.96 + 78    # 3-plane C add
        if k % 2 == 0:
            pw = float(slotw[k] + slotw[k + 1]) if k + 1 < S else vk
            act += (172 + pw) / 1.2 + 60        # sigmoid pair PSUM->SBUF
    cost = {
        "dve": lambda vk: (58 + vk / 4) / 0.96 + 50,
        "act": lambda vk: (224 + vk) / 1.2 + 40,
        "pool": lambda vk: 260 + vk * 1.1 / 1.2,
    }
    busy = {"dve": dve, "act": act, "pool": pool}
    assign = []  # per slot: tuple of 3 engine names (for channels R,G,B)
    for k in range(S):
        vk = float(slotw[k])
        chans = []
        for _ in range(3):
            e = min(busy, key=lambda e: busy[e] + cost[e](vk))
            busy[e] += cost[e](vk)
            chans.append(e)
        assign.append(tuple(chans))
    return assign, busy


# ------------------------------------------------------------- device build
def build_nc(slotw):
    slotw = [int(v) for v in slotw]
    S = len(slotw)
    npairs = S // 2
    pairw = [slotw[2 * i] + slotw[2 * i + 1] for i in range(npairs)]
    pair_start = [0]
    for pw in pairw:
        pair_start.append(pair_start[-1] + pw)
    rhs_len = pair_start[-1]
    ngroups = (S + 7) // 8
    assign, _busy = _assign_premults(slotw)

    nc = bacc.Bacc("TRN2", target_bir_lowering=False, debug=False,
                   num_devices=N_CORES)
    scal_d = nc.dram_tensor("scal", [ROWS, S * 4], F32,
                            kind="ExternalInput").ap()
    offs_d = nc.dram_tensor("offs", [1, S], I32, kind="ExternalInput").ap()
    lhsT_d = nc.dram_tensor("lhsT", [8, npairs * ROWS], BF16,
                            kind="ExternalInput").ap()
    rhs_d = nc.dram_tensor("rhs", [8, rhs_len], BF16,
                           kind="ExternalInput").ap()
    out_d = nc.dram_tensor("out", [ROWS, 4 * W], F16,
                           kind="ExternalOutput").ap()

    with TileContext(nc) as tc:
        T = nc.alloc_sbuf_tensor("T", [ROWS, W], F16).ap()
        CC = nc.alloc_sbuf_tensor("CC", [ROWS, 3 * W], F16).ap()
        AT = nc.alloc_sbuf_tensor("AT", [ROWS, W], F16).ap()
        covr = nc.alloc_sbuf_tensor("covr", [ROWS, 2 * 1792], F16).ap()
        wr = nc.alloc_sbuf_tensor("wr", [ROWS, 8 * WCAP], F16).ap()
        mr = nc.alloc_sbuf_tensor("mr", [ROWS, 4 * 3 * WCAP], F16).ap()
        scal_sb = nc.alloc_sbuf_tensor("scal_sb", [ROWS, S * 4], F32).ap()
        offs_sb = nc.alloc_sbuf_tensor("offs_sb", [1, S], I32).ap()

        nc.vector.memset(T, 1.0)
        nc.gpsimd.memset(CC, 0.0)

        CC3 = CC.rearrange("p (c x) -> p c x", x=W)
        mr3 = mr.rearrange("p (s x) -> p s x", x=WCAP)

        with (
            tc.tile_pool(name="psum", bufs=2, space="PSUM") as psum_pool,
            tc.tile_pool(name="ops", bufs=3) as oppool,
        ):
            pend = []  # [(slot k, width, offset-reg)] awaiting C adds
            for g in range(ngroups):
                k0 = g * 8
                p0 = k0 // 2
                gsize = min(8, S - k0)
                gp = gsize // 2
                gw = pair_start[p0 + gp] - pair_start[p0]
                lh_t = oppool.tile([8, 4 * ROWS], BF16, tag="lh")
                rh_t = oppool.tile([8, 1792], BF16, tag="rh")
                nc.sync.dma_start(lh_t[:, :gp * ROWS],
                                  lhsT_d[:, p0 * ROWS:(p0 + gp) * ROWS])
                nc.sync.dma_start(rh_t[:, :gw],
                                  rhs_d[:, pair_start[p0]:pair_start[p0 + gp]])
                if g == 0:
                    nc.sync.dma_start(offs_sb, offs_d)
                    nc.sync.dma_start(scal_sb, scal_d)
                pt = psum_pool.tile([ROWS, 4 * 512], F32)
                cbase = (g % 2) * 1792
                rpos = 0
                pair_pos = []
                for i in range(gp):
                    pw = pairw[p0 + i]
                    pair_pos.append(rpos)
                    nc.tensor.matmul(
                        pt[:, i * 512:i * 512 + pw],
                        lh_t[:, i * ROWS:(i + 1) * ROWS],
                        rh_t[:, rpos:rpos + pw],
                        start=True, stop=True)
                    rpos += pw

                # dynamic offsets for the group (DVE only)
                vregs = [nc.vector.alloc_register(f"voff_{k0}_{i}")
                         for i in range(gsize)]
                nc.vector.reg_load(vregs, offs_sb[0:1, k0:k0 + gsize])
                voff = [nc.vector.snap(vregs[j], donate=True, min_val=0,
                                       max_val=W - slotw[k0 + j])
                        for j in range(gsize)]

                rpos = 0
                for j in range(gsize):
                    k = k0 + j
                    vk = slotw[k]
                    if j % 2 == 0:
                        i = j // 2
                        pw = pairw[p0 + i]
                        nc.scalar.activation(
                            covr[:, cbase + pair_pos[i]:
                                 cbase + pair_pos[i] + pw],
                            pt[:, i * 512:i * 512 + pw], AF.Sigmoid)
                    cov = covr[:, cbase + rpos:cbase + rpos + vk]
                    rpos += vk
                    wv = wr[:, (k % 8) * WCAP:(k % 8) * WCAP + vk]
                    tw = T[:, bass.ds(voff[j], vk)]
                    # DVE chain: w = cov*T ; T = (-a)*w + T
                    nc.vector.tensor_tensor(wv, cov, tw, OP.mult)
                    nc.vector.scalar_tensor_tensor(
                        tw, wv, scal_sb[:, k * 4:k * 4 + 1], tw,
                        OP.mult, OP.add)
                    # premults m_ch = (a col_ch) * w, engine per balancer
                    for ch in range(3):
                        mb = ((k % 4) * 3 + ch) * WCAP
                        sc = scal_sb[:, k * 4 + 1 + ch:k * 4 + 2 + ch]
                        eng = assign[k][ch]
                        if eng == "dve":
                            nc.vector.tensor_scalar(
                                mr[:, mb:mb + vk], wv, sc, 0.0,
                                OP.mult, OP.add)
                        elif eng == "pool":
                            nc.gpsimd.tensor_scalar(
                                mr[:, mb:mb + vk], wv, sc, 0.0,
                                OP.mult, OP.add)
                        else:
                            nc.scalar.activation(
                                mr[:, mb:mb + vk], wv, AF.Copy, scale=sc)
                    # delayed batched C add (2 slots behind)
                    pend.append((k, vk, voff[j]))
                    if len(pend) > 2:
                        kp, vp, offp = pend.pop(0)
                        qv = CC3[:, :, bass.ds(offp, vp)]
                        mp = mr3[:, (kp % 4) * 3:(kp % 4) * 3 + 3, :vp]
                        nc.vector.tensor_tensor(qv, qv, mp, OP.add)

            for kp, vp, offp in pend:
                qv = CC3[:, :, bass.ds(offp, vp)]
                mp = mr3[:, (kp % 4) * 3:(kp % 4) * 3 + 3, :vp]
                nc.vector.tensor_tensor(qv, qv, mp, OP.add)
            pend = []

        # A = 1 - T, then plane DMAs
        nc.vector.tensor_scalar(AT, T, -1.0, 1.0, OP.mult, OP.add)
        nc.sync.dma_start(out_d[:, 0:3 * W], CC)
        nc.sync.dma_start(out_d[:, 3 * W:4 * W], AT)

    nc.compile()
    return nc


_CACHE = {}


def _get_nc(slotw):
    key = tuple(int(v) for v in slotw)
    if key not in _CACHE:
        _CACHE[key] = build_nc(slotw)
    return _CACHE[key]


def kernel(centers, radii, colors):
    centers = np.asarray(centers, np.float32)
    radii = np.asarray(radii, np.float32)
    colors = np.asarray(colors, np.float32)

    plan = make_plan(centers, radii)
    nc = _get_nc(plan[1])
    ins = make_inputs(centers, radii, colors, plan)
    res = bass_utils.run_bass_kernel_spmd(nc, ins, list(range(N_CORES)),
                                          trace=False)
    out = np.empty((H, W, 4), np.float32)
    for c in range(N_CORES):
        planes = res.results[c]["out"].astype(np.float32)  # [128, 4*W]
        for ch in range(4):
            out[c * ROWS:(c + 1) * ROWS, :, ch] = planes[:, ch * W:(ch + 1) * W]
    return out


# revision 3
# speedup vs baseline: 1.7472x; 1.7472x over previous
"""DiffVG-style circle renderer on 8 Trainium2 NeuronCores.

v3: per-core specialized programs via an 8-way tc.Switch on partition_id().
Each arm is fully static for its core: window widths/offsets are
compile-time constants, per-circle scalars (alpha, alpha*color) are
instruction immediates. This removes the shared cross-core max-width
padding (-25% elements), the dynamic-offset register machinery
(~140ns/slot on DVE), and the per-partition AP-scalar reads
(+60..150ns/op).

Per core: shard image by rows (128 rows/core); composite the circles
intersecting the band front-to-back (descending z) with transmittance T:
    w   = cov * T        (DVE TT 2x, cov = sigmoid(2(r-d)) from ACT/PSUM)
    T  += (-a) * w       (DVE stt, immediate scalar — chain stays on DVE)
    m_ch = (a col_ch) w  (premult, spread over ACT/Pool/DVE, immediates)
    C3  += m3            (DVE 3-plane TT, delayed 2 slots)
PE computes z=(r^2-d^2)/r per circle-pair as a K=8 outer-sum matmul with
bf16 hi/lo split operands. Output: [T->A, C_RGB] fp16 planes DMA'd out,
assembled to f32 on host.
"""

import sys

if "/opt/trn_rl_repo" not in sys.path:
    sys.path.insert(0, "/opt/trn_rl_repo")

import numpy as np
import ml_dtypes

import concourse.bass as bass
import concourse.bacc as bacc
import concourse.mybir as mybir
from concourse.tile import TileContext
from concourse import bass_utils

H = 1024
W = 1024
ROWS = 128
N_CORES = 8
MARGIN = 5.0
ROUND = 8
WMIN = 16
WCAP = 224
F32 = mybir.dt.float32
F16 = mybir.dt.float16
BF16 = mybir.dt.bfloat16
AF = mybir.ActivationFunctionType
OP = mybir.AluOpType
BF = ml_dtypes.bfloat16


# ---------------------------------------------------------------- host plan
def _core_circles(centers, radii, core):
    """Kept circle indices (descending z = front-to-back) + widths/offsets."""
    y0 = ROWS * core
    cy = centers[:, 1].astype(np.float64)
    cx = centers[:, 0].astype(np.float64)
    r = radii.astype(np.float64)
    keep = (cy + r + MARGIN >= y0 + 0.5) & (cy - r - MARGIN <= y0 + ROWS - 0.5)
    idx = np.where(keep)[0][::-1]  # descending index = front-to-back
    dymin = np.maximum(0.0, np.maximum(y0 + 0.5 - cy[idx],
                                       cy[idx] - (y0 + ROWS - 0.5)))
    rm = r[idx] + MARGIN
    halfw = np.sqrt(np.maximum(rm * rm - dymin * dymin, 4.0))
    ws = np.clip(np.ceil(2.0 * halfw / ROUND) * ROUND, WMIN, WCAP).astype(int)
    off = np.clip(np.round(cx[idx] - ws / 2.0), 0, W - ws).astype(int)
    return idx, ws, off


def make_plan(centers, radii, colors):
    """Per-core plan dicts with all compile-time constants."""
    col = colors.astype(np.float64)
    r = radii.astype(np.float64)
    cx = centers[:, 0].astype(np.float64)
    cy = centers[:, 1].astype(np.float64)
    plans = []
    for core in range(N_CORES):
        ids, ws, off = _core_circles(centers, radii, core)
        n = len(ids)
        npairs = (n + 1) // 2
        pairw = []
        for p in range(npairs):
            w0 = int(ws[2 * p])
            w1 = int(ws[2 * p + 1]) if 2 * p + 1 < n else 0
            pairw.append(w0 + w1)
        plans.append({
            "core": core, "ids": ids, "ws": ws.astype(int), "off": off,
            "n": n, "npairs": npairs, "pairw": pairw,
            "alpha": col[ids, 3], "colr": col[ids, 0], "colg": col[ids, 1],
            "colb": col[ids, 2],
            "r": r[ids], "cx": cx[ids], "cy": cy[ids],
        })
    return plans


def _hilo(x):
    hi = x.astype(BF)
    lo = (x - hi.astype(np.float64)).astype(BF)
    return hi, lo


def make_inputs(plans):
    npairs_max = max(p["npairs"] for p in plans)
    rhs_max = max(sum(p["pairw"]) for p in plans)
    ins = []
    for p in plans:
        core = p["core"]
        y0 = ROWS * core
        n = p["n"]
        lhsT = np.zeros((8, npairs_max * ROWS), BF)
        rhs = np.zeros((8, rhs_max), BF)
        rows = y0 + np.arange(ROWS, dtype=np.float64) + 0.5
        rpos = 0
        for k in range(n):
            vk = int(p["ws"][k])
            off = int(p["off"][k])
            ri, cxi, cyi = p["r"][k], p["cx"][k], p["cy"][k]
            j = off + np.arange(vk, dtype=np.float64) + 0.5
            a = ri / 2.0 - (rows - cyi) ** 2 / ri
            b = ri / 2.0 - (j - cxi) ** 2 / ri
            ah, alo = _hilo(a)
            bh, blo = _hilo(b)
            pair, half = divmod(k, 2)
            rb = 4 * half
            ls = slice(pair * ROWS, (pair + 1) * ROWS)
            lhsT[rb + 0, ls] = ah
            lhsT[rb + 1, ls] = alo
            lhsT[rb + 2, ls] = 1.0
            lhsT[rb + 3, ls] = 1.0
            if half == 0:
                rpos_pair = rpos
                rpos += p["pairw"][pair]
            c0 = rpos_pair + (0 if half == 0 else int(p["ws"][2 * pair]))
            rs = slice(c0, c0 + vk)
            rhs[rb + 0, rs] = 1.0
            rhs[rb + 1, rs] = 1.0
            rhs[rb + 2, rs] = bh
            rhs[rb + 3, rs] = blo
        ins.append({"lhsT": lhsT, "rhs": rhs})
    return ins, npairs_max, rhs_max


# ------------------------------------------------- premult engine balancer
def _assign_premults(ws, npairs, pairw):
    """Greedy per-core assignment of 3 premult channels per slot to
    ACT/Pool/DVE, balancing projected busy (ns, immediates)."""
    n = len(ws)
    dve = act = pool = 0.0
    for k in range(n):
        vk = float(ws[k])
        dve += (58 + vk / 2) / 0.96 + 78          # TT w'
        dve += (58 + vk) / 0.96 + 10              # stt imm
        dve += (58 + 3 * vk / 2) / 0.96 + 78      # 3-plane C add
    for pw in pairw:
        act += (172 + pw) / 1.2 + 60              # sigmoid PSUM->SBUF
    cost = {
        "dve": lambda vk: (58 + vk / 4) / 0.96 + 50,
        "act": lambda vk: (224 + vk) / 1.2 + 40,
        "pool": lambda vk: 210 + vk * 1.2 / 1.2,
    }
    busy = {"dve": dve, "act": act, "pool": pool}
    assign = []
    for k in range(n):
        vk = float(ws[k])
        chans = []
        for _ in range(3):
            e = min(busy, key=lambda e: busy[e] + cost[e](vk))
            busy[e] += cost[e](vk)
            chans.append(e)
        assign.append(tuple(chans))
    return assign


# ------------------------------------------------------------- device build
def build_nc(plans, npairs_max, rhs_max):
    nc = bacc.Bacc("TRN2", target_bir_lowering=False, debug=False,
                   num_devices=N_CORES)
    lhsT_d = nc.dram_tensor("lhsT", [8, npairs_max * ROWS], BF16,
                            kind="ExternalInput").ap()
    rhs_d = nc.dram_tensor("rhs", [8, rhs_max], BF16,
                           kind="ExternalInput").ap()
    out_d = nc.dram_tensor("out", [ROWS, 4 * W], F16,
                           kind="ExternalOutput").ap()

    with TileContext(nc) as tc:
        T = nc.alloc_sbuf_tensor("T", [ROWS, W], F16).ap()
        CC = nc.alloc_sbuf_tensor("CC", [ROWS, 3 * W], F16).ap()
        AT = nc.alloc_sbuf_tensor("AT", [ROWS, W], F16).ap()
        covr = nc.alloc_sbuf_tensor("covr", [ROWS, 2 * 1792], F16).ap()
        wr = nc.alloc_sbuf_tensor("wr", [ROWS, 8 * WCAP], F16).ap()
        mr = nc.alloc_sbuf_tensor("mr", [ROWS, 8 * 3 * WCAP], F16).ap()
        lh_sb = nc.alloc_sbuf_tensor("lh_sb", [8, 2 * 4 * ROWS], BF16).ap()
        rh_sb = nc.alloc_sbuf_tensor("rh_sb", [8, 2 * 1792], BF16).ap()
        pt0 = nc.alloc_psum_tensor("pt0", [ROWS, 4 * 512], F32).ap()
        pt1 = nc.alloc_psum_tensor("pt1", [ROWS, 4 * 512], F32).ap()
        pts = [pt0, pt1]

        nc.vector.memset(T, 1.0)
        nc.gpsimd.memset(CC, 0.0)

        CC3 = CC.rearrange("p (c x) -> p c x", x=W)
        mr3 = mr.rearrange("p (s x) -> p s x", x=WCAP)

        pid = nc.partition_id()

        for core in tc.Switch(pid, N_CORES):
            p = plans[core]
            n = p["n"]
            ws, off = p["ws"], p["off"]
            pairw = p["pairw"]
            pair_start = np.concatenate([[0], np.cumsum(pairw)]).astype(int)
            assign = _assign_premults(ws, p["npairs"], pairw)
            ngroups = (n + 7) // 8
            pend = []
            for g in range(ngroups):
                k0 = g * 8
                gsize = min(8, n - k0)
                gp = (gsize + 1) // 2
                p0 = k0 // 2
                gw = int(pair_start[p0 + gp] - pair_start[p0])
                lh = lh_sb[:, (g % 2) * 4 * ROWS:]
                rh = rh_sb[:, (g % 2) * 1792:]
                nc.sync.dma_start(lh[:, :gp * ROWS],
                                  lhsT_d[:, p0 * ROWS:(p0 + gp) * ROWS])
                nc.sync.dma_start(rh[:, :gw],
                                  rhs_d[:, int(pair_start[p0]):
                                        int(pair_start[p0 + gp])])
                pt = pts[g % 2]
                cbase = (g % 2) * 1792
                rpos = 0
                pair_pos = []
                for i in range(gp):
                    pw = pairw[p0 + i]
                    pair_pos.append(rpos)
                    nc.tensor.matmul(
                        pt[:, i * 512:i * 512 + pw],
                        lh[:, i * ROWS:(i + 1) * ROWS],
                        rh[:, rpos:rpos + pw],
                        start=True, stop=True)
                    rpos += pw

                rpos = 0
                for j in range(gsize):
                    k = k0 + j
                    vk = int(ws[k])
                    ok = int(off[k])
                    if j % 2 == 0:
                        i = j // 2
                        pw = pairw[p0 + i]
                        nc.scalar.activation(
                            covr[:, cbase + pair_pos[i]:
                                 cbase + pair_pos[i] + pw],
                            pt[:, i * 512:i * 512 + pw], AF.Sigmoid)
                    cov = covr[:, cbase + rpos:cbase + rpos + vk]
                    rpos += vk
                    wv = wr[:, (k % 8) * WCAP:(k % 8) * WCAP + vk]
                    tw = T[:, ok:ok + vk]
                    al = float(p["alpha"][k])
                    acr = float(p["alpha"][k] * p["colr"][k])
                    acg = float(p["alpha"][k] * p["colg"][k])
                    acb = float(p["alpha"][k] * p["colb"][k])
                    # DVE chain: w = cov*T ; T = (-a)*w + T
                    nc.vector.tensor_tensor(wv, cov, tw, OP.mult)
                    nc.vector.scalar_tensor_tensor(tw, wv, -al, tw,
                                                   OP.mult, OP.add)
                    # premults m_ch = (a col_ch) * w, engine per balancer
                    for ch, sc in enumerate((acr, acg, acb)):
                        mb = ((k % 8) * 3 + ch) * WCAP
                        eng = assign[k][ch]
                        if eng == "dve":
                            nc.vector.tensor_scalar(
                                mr[:, mb:mb + vk], wv, sc, 0.0,
                                OP.mult, OP.add)
                        elif eng == "pool":
                            nc.gpsimd.tensor_scalar(
                                mr[:, mb:mb + vk], wv, sc, 0.0,
                                OP.mult, OP.add)
                        else:
                            nc.scalar.activation(
                                mr[:, mb:mb + vk], wv, AF.Copy, scale=sc)
                    # delayed batched C add (2 slots behind)
                    pend.append((k, vk, ok))
                    if len(pend) > 2:
                        kp, vp, op_ = pend.pop(0)
                        qv = CC3[:, :, op_:op_ + vp]
                        mp = mr3[:, (kp % 8) * 3:(kp % 8) * 3 + 3, :vp]
                        nc.vector.tensor_tensor(qv, qv, mp, OP.add)
            for kp, vp, op_ in pend:
                qv = CC3[:, :, op_:op_ + vp]
                mp = mr3[:, (kp % 8) * 3:(kp % 8) * 3 + 3, :vp]
                nc.vector.tensor_tensor(qv, qv, mp, OP.add)

        # A = 1 - T, then plane DMAs
        nc.vector.tensor_scalar(AT, T, -1.0, 1.0, OP.mult, OP.add)
        nc.sync.dma_start(out_d[:, 0:3 * W], CC)
        nc.sync.dma_start(out_d[:, 3 * W:4 * W], AT)

    nc.compile()
    return nc


def kernel(centers, radii, colors):
    centers = np.asarray(centers, np.float32)
    radii = np.asarray(radii, np.float32)
    colors = np.asarray(colors, np.float32)

    plans = make_plan(centers, radii, colors)
    ins, npairs_max, rhs_max = make_inputs(plans)
    nc = build_nc(plans, npairs_max, rhs_max)
    res = bass_utils.run_bass_kernel_spmd(nc, ins, list(range(N_CORES)),
                                          trace=False)
    out = np.empty((H, W, 4), np.float32)
    for c in range(N_CORES):
        planes = res.results[c]["out"].astype(np.float32)  # [128, 4*W]
        for ch in range(4):
            out[c * ROWS:(c + 1) * ROWS, :, ch] = planes[:, ch * W:(ch + 1) * W]
    return out


# revision 7
# speedup vs baseline: 1.7694x; 1.0127x over previous
"""DiffVG-style circle renderer on 8 Trainium2 NeuronCores.

v3: per-core specialized programs via an 8-way tc.Switch on partition_id().
Each arm is fully static for its core: window widths/offsets are
compile-time constants, per-circle scalars (alpha, alpha*color) are
instruction immediates. This removes the shared cross-core max-width
padding (-25% elements), the dynamic-offset register machinery
(~140ns/slot on DVE), and the per-partition AP-scalar reads
(+60..150ns/op).

Per core: shard image by rows (128 rows/core); composite the circles
intersecting the band front-to-back (descending z) with transmittance T:
    w   = cov * T        (DVE TT 2x, cov = sigmoid(2(r-d)) from ACT/PSUM)
    T  += (-a) * w       (DVE stt, immediate scalar — chain stays on DVE)
    m_ch = (a col_ch) w  (premult, spread over ACT/Pool/DVE, immediates)
    C3  += m3            (DVE 3-plane TT, delayed 2 slots)
PE computes z=(r^2-d^2)/r per circle-pair as a K=8 outer-sum matmul with
bf16 hi/lo split operands. Output: [T->A, C_RGB] fp16 planes DMA'd out,
assembled to f32 on host.
"""

import sys

if "/opt/trn_rl_repo" not in sys.path:
    sys.path.insert(0, "/opt/trn_rl_repo")

import numpy as np
import ml_dtypes

import concourse.bass as bass
import concourse.bacc as bacc
import concourse.mybir as mybir
from concourse.tile import TileContext
from concourse import bass_utils

H = 1024
W = 1024
ROWS = 128
N_CORES = 8
MARGIN = 5.0
ROUND = 8
WMIN = 16
WCAP = 224
F32 = mybir.dt.float32
F16 = mybir.dt.float16
BF16 = mybir.dt.bfloat16
AF = mybir.ActivationFunctionType
OP = mybir.AluOpType
BF = ml_dtypes.bfloat16


# ---------------------------------------------------------------- host plan
def _core_circles(centers, radii, core):
    """Kept circle indices (descending z = front-to-back) + widths/offsets."""
    y0 = ROWS * core
    cy = centers[:, 1].astype(np.float64)
    cx = centers[:, 0].astype(np.float64)
    r = radii.astype(np.float64)
    keep = (cy + r + MARGIN >= y0 + 0.5) & (cy - r - MARGIN <= y0 + ROWS - 0.5)
    idx = np.where(keep)[0][::-1]  # descending index = front-to-back
    dymin = np.maximum(0.0, np.maximum(y0 + 0.5 - cy[idx],
                                       cy[idx] - (y0 + ROWS - 0.5)))
    rm = r[idx] + MARGIN
    halfw = np.sqrt(np.maximum(rm * rm - dymin * dymin, 4.0))
    ws = np.clip(np.ceil(2.0 * halfw / ROUND) * ROUND, WMIN, WCAP).astype(int)
    off = np.clip(np.round(cx[idx] - ws / 2.0), 0, W - ws).astype(int)
    return idx, ws, off


def make_plan(centers, radii, colors):
    """Per-core plan dicts with all compile-time constants."""
    col = colors.astype(np.float64)
    r = radii.astype(np.float64)
    cx = centers[:, 0].astype(np.float64)
    cy = centers[:, 1].astype(np.float64)
    plans = []
    for core in range(N_CORES):
        ids, ws, off = _core_circles(centers, radii, core)
        n = len(ids)
        npairs = (n + 1) // 2
        pairw = []
        for p in range(npairs):
            w0 = int(ws[2 * p])
            w1 = int(ws[2 * p + 1]) if 2 * p + 1 < n else 0
            pairw.append(w0 + w1)
        plans.append({
            "core": core, "ids": ids, "ws": ws.astype(int), "off": off,
            "n": n, "npairs": npairs, "pairw": pairw,
            "alpha": col[ids, 3], "colr": col[ids, 0], "colg": col[ids, 1],
            "colb": col[ids, 2],
            "r": r[ids], "cx": cx[ids], "cy": cy[ids],
        })
    return plans


def _hilo(x):
    hi = x.astype(BF)
    lo = (x - hi.astype(np.float64)).astype(BF)
    return hi, lo


def make_inputs(plans):
    """rhs is group-strided: group g's 4 pairs are packed from DRAM offset
    g*1792 (pair_pos = within-group cumsum), so every core's group-g DMA
    reads the same DRAM range."""
    npairs_max = max(p["npairs"] for p in plans)
    ngroups_max = max((p["n"] + 7) // 8 for p in plans)
    rhs_max = ngroups_max * 1792
    ins = []
    for p in plans:
        core = p["core"]
        y0 = ROWS * core
        n = p["n"]
        lhsT = np.zeros((8, npairs_max * ROWS), BF)
        rhs = np.zeros((8, rhs_max), BF)
        rows = y0 + np.arange(ROWS, dtype=np.float64) + 0.5
        for k in range(n):
            vk = int(p["ws"][k])
            off = int(p["off"][k])
            ri, cxi, cyi = p["r"][k], p["cx"][k], p["cy"][k]
            j = off + np.arange(vk, dtype=np.float64) + 0.5
            a = ri / 2.0 - (rows - cyi) ** 2 / ri
            b = ri / 2.0 - (j - cxi) ** 2 / ri
            ah, alo = _hilo(a)
            bh, blo = _hilo(b)
            pair, half = divmod(k, 2)
            rb = 4 * half
            ls = slice(pair * ROWS, (pair + 1) * ROWS)
            lhsT[rb + 0, ls] = ah
            lhsT[rb + 1, ls] = alo
            lhsT[rb + 2, ls] = 1.0
            lhsT[rb + 3, ls] = 1.0
            g, ip = divmod(pair, 4)
            c0 = g * 1792 + sum(p["pairw"][4 * g:pair]) \
                + (0 if half == 0 else int(p["ws"][2 * pair]))
            rs = slice(c0, c0 + vk)
            rhs[rb + 0, rs] = 1.0
            rhs[rb + 1, rs] = 1.0
            rhs[rb + 2, rs] = bh
            rhs[rb + 3, rs] = blo
        ins.append({"lhsT": lhsT, "rhs": rhs})
    return ins, npairs_max, rhs_max


# ------------------------------------------------- premult engine balancer
def _assign_premults(ws, npairs, pairw):
    """Greedy per-core assignment of each slot's 3 premult channels to ONE
    engine (ACT/Pool/DVE) — single producer per slot keeps the C-add's
    cross-engine sync to one semaphore. Costs in ns, immediates."""
    n = len(ws)
    dve = act = pool = 0.0
    for k in range(n):
        vk = float(ws[k])
        dve += (58 + vk / 2) / 0.96 + 78          # TT w'
        dve += (58 + vk) / 0.96 + 150             # stt imm
        dve += (58 + 3 * vk / 2) / 0.96 + 78      # 3-plane C add
    for pw in pairw:
        act += (172 + pw) / 1.2 + 60              # sigmoid PSUM->SBUF
    cost = {
        "dve": lambda vk: 3 * ((58 + vk / 4) / 0.96 + 50),
        "act": lambda vk: 3 * ((224 + vk) / 1.2 + 40),
        "pool": lambda vk: 3 * (210 + vk * 1.2 / 1.2),
    }
    busy = {"dve": dve, "act": act, "pool": pool}
    assign = []
    for k in range(n):
        vk = float(ws[k])
        e = min(busy, key=lambda e: busy[e] + cost[e](vk))
        busy[e] += cost[e](vk)
        assign.append((e, e, e))
    return assign


# ------------------------------------------------------------- device build
def build_nc(plans, npairs_max, rhs_max):
    nc = bacc.Bacc("TRN2", target_bir_lowering=False, debug=False,
                   num_devices=N_CORES)
    lhsT_d = nc.dram_tensor("lhsT", [8, npairs_max * ROWS], BF16,
                            kind="ExternalInput").ap()
    rhs_d = nc.dram_tensor("rhs", [8, rhs_max], BF16,
                           kind="ExternalInput").ap()
    out_d = nc.dram_tensor("out", [ROWS, 4 * W], F16,
                           kind="ExternalOutput").ap()

    with TileContext(nc) as tc:
        T = nc.alloc_sbuf_tensor("T", [ROWS, W], F16).ap()
        CC = nc.alloc_sbuf_tensor("CC", [ROWS, 3 * W], F16).ap()
        AT = nc.alloc_sbuf_tensor("AT", [ROWS, W], F16).ap()
        covr = nc.alloc_sbuf_tensor("covr", [ROWS, 2 * 1792], F16).ap()
        wr = nc.alloc_sbuf_tensor("wr", [ROWS, 8 * WCAP], F16).ap()
        mr = nc.alloc_sbuf_tensor("mr", [ROWS, 8 * 3 * WCAP], F16).ap()
        lh_sb = nc.alloc_sbuf_tensor("lh_sb", [8, 2 * 4 * ROWS], BF16).ap()
        rh_sb = nc.alloc_sbuf_tensor("rh_sb", [8, 2 * 1792], BF16).ap()
        pt0 = nc.alloc_psum_tensor("pt0", [ROWS, 4 * 512], F32).ap()
        pt1 = nc.alloc_psum_tensor("pt1", [ROWS, 4 * 512], F32).ap()
        pts = [pt0, pt1]

        nc.vector.memset(T, 1.0)
        nc.gpsimd.memset(CC, 0.0)

        CC3 = CC.rearrange("p (c x) -> p c x", x=W)
        mr3 = mr.rearrange("p (s x) -> p s x", x=WCAP)

        pid = nc.partition_id()
        from concourse.expressions import s_valid_engines
        hint = tc.switch_hint({e: pid for e in s_valid_engines(pid)},
                              N_CORES, label="corearm")

        # prefetch the first two groups' operands before the dispatch —
        # group-strided DRAM layout makes the ranges identical on all cores
        ngroups_max = rhs_max // 1792
        for g in (0, 1):
            if g * 4 * ROWS < npairs_max * ROWS:
                hi = min((g + 1) * 4 * ROWS, npairs_max * ROWS)
                nc.sync.dma_start(
                    lh_sb[:, (g % 2) * 4 * ROWS:(g % 2) * 4 * ROWS
                          + hi - g * 4 * ROWS],
                    lhsT_d[:, g * 4 * ROWS:hi])
            if g < ngroups_max:
                nc.sync.dma_start(rh_sb[:, (g % 2) * 1792:(g % 2 + 1) * 1792],
                                  rhs_d[:, g * 1792:(g + 1) * 1792])

        for core in tc.Switch(pid, N_CORES, hint=hint):
            p = plans[core]
            n = p["n"]
            ws, off = p["ws"], p["off"]
            pairw = p["pairw"]
            assign = _assign_premults(ws, p["npairs"], pairw)
            ngroups = (n + 7) // 8
            pend = []
            for g in range(ngroups):
                k0 = g * 8
                gsize = min(8, n - k0)
                gp = (gsize + 1) // 2
                p0 = k0 // 2
                lh = lh_sb[:, (g % 2) * 4 * ROWS:]
                rh = rh_sb[:, (g % 2) * 1792:]
                if g >= 2:
                    nc.sync.dma_start(lh[:, :gp * ROWS],
                                      lhsT_d[:, p0 * ROWS:(p0 + gp) * ROWS])
                    nc.sync.dma_start(rh[:, :1792],
                                      rhs_d[:, g * 1792:(g + 1) * 1792])
                pt = pts[g % 2]
                cbase = (g % 2) * 1792
                rpos = 0
                pair_pos = []
                for i in range(gp):
                    pw = pairw[p0 + i]
                    pair_pos.append(rpos)
                    nc.tensor.matmul(
                        pt[:, i * 512:i * 512 + pw],
                        lh[:, i * ROWS:(i + 1) * ROWS],
                        rh[:, rpos:rpos + pw],
                        start=True, stop=True)
                    rpos += pw

                rpos = 0
                for j in range(gsize):
                    k = k0 + j
                    vk = int(ws[k])
                    ok = int(off[k])
                    if j % 2 == 0:
                        i = j // 2
                        pw = pairw[p0 + i]
                        nc.scalar.activation(
                            covr[:, cbase + pair_pos[i]:
                                 cbase + pair_pos[i] + pw],
                            pt[:, i * 512:i * 512 + pw], AF.Sigmoid)
                    cov = covr[:, cbase + rpos:cbase + rpos + vk]
                    rpos += vk
                    wv = wr[:, (k % 8) * WCAP:(k % 8) * WCAP + vk]
                    tw = T[:, ok:ok + vk]
                    al = float(p["alpha"][k])
                    acr = float(p["alpha"][k] * p["colr"][k])
                    acg = float(p["alpha"][k] * p["colg"][k])
                    acb = float(p["alpha"][k] * p["colb"][k])
                    # DVE chain: w = cov*T ; T = (-a)*w + T
                    nc.vector.tensor_tensor(wv, cov, tw, OP.mult)
                    nc.vector.scalar_tensor_tensor(tw, wv, -al, tw,
                                                   OP.mult, OP.add)
                    # premults m_ch = (a col_ch) * w, engine per balancer
                    for ch, sc in enumerate((acr, acg, acb)):
                        mb = ((k % 8) * 3 + ch) * WCAP
                        eng = assign[k][ch]
                        if eng == "dve":
                            nc.vector.tensor_scalar(
                                mr[:, mb:mb + vk], wv, sc, 0.0,
                                OP.mult, OP.add)
                        elif eng == "pool":
                            nc.gpsimd.tensor_scalar(
                                mr[:, mb:mb + vk], wv, sc, 0.0,
                                OP.mult, OP.add)
                        else:
                            nc.scalar.activation(
                                mr[:, mb:mb + vk], wv, AF.Copy, scale=sc)
                    # delayed batched C add (2 slots behind)
                    pend.append((k, vk, ok))
                    if len(pend) > 2:
                        kp, vp, op_ = pend.pop(0)
                        qv = CC3[:, :, op_:op_ + vp]
                        mp = mr3[:, (kp % 8) * 3:(kp % 8) * 3 + 3, :vp]
                        nc.vector.tensor_tensor(qv, qv, mp, OP.add)
            for kp, vp, op_ in pend:
                qv = CC3[:, :, op_:op_ + vp]
                mp = mr3[:, (kp % 8) * 3:(kp % 8) * 3 + 3, :vp]
                nc.vector.tensor_tensor(qv, qv, mp, OP.add)

        # A = 1 - T, then plane DMAs
        nc.vector.tensor_scalar(AT, T, -1.0, 1.0, OP.mult, OP.add)
        nc.sync.dma_start(out_d[:, 0:3 * W], CC)
        nc.sync.dma_start(out_d[:, 3 * W:4 * W], AT)

    nc.compile()
    return nc


def kernel(centers, radii, colors):
    centers = np.asarray(centers, np.float32)
    radii = np.asarray(radii, np.float32)
    colors = np.asarray(colors, np.float32)

    plans = make_plan(centers, radii, colors)
    ins, npairs_max, rhs_max = make_inputs(plans)
    nc = build_nc(plans, npairs_max, rhs_max)
    res = bass_utils.run_bass_kernel_spmd(nc, ins, list(range(N_CORES)),
                                          trace=False)
    out = np.empty((H, W, 4), np.float32)
    for c in range(N_CORES):
        planes = res.results[c]["out"].astype(np.float32)  # [128, 4*W]
        for ch in range(4):
            out[c * ROWS:(c + 1) * ROWS, :, ch] = planes[:, ch * W:(ch + 1) * W]
    return out


# revision 13
# speedup vs baseline: 1.8406x; 1.0402x over previous
"""DiffVG-style circle renderer on 8 Trainium2 NeuronCores.

v3: per-core specialized programs via an 8-way tc.Switch on partition_id().
Each arm is fully static for its core: window widths/offsets are
compile-time constants, per-circle scalars (alpha, alpha*color) are
instruction immediates. This removes the shared cross-core max-width
padding (-25% elements), the dynamic-offset register machinery
(~140ns/slot on DVE), and the per-partition AP-scalar reads
(+60..150ns/op).

Per core: shard image by rows (128 rows/core); composite the circles
intersecting the band front-to-back (descending z) with transmittance T:
    w   = cov * T        (DVE TT 2x, cov = sigmoid(2(r-d)) from ACT/PSUM)
    T  += (-a) * w       (DVE stt, immediate scalar — chain stays on DVE)
    m_ch = (a col_ch) w  (premult, spread over ACT/Pool/DVE, immediates)
    C3  += m3            (DVE 3-plane TT, delayed 2 slots)
PE computes z=(r^2-d^2)/r per circle-pair as a K=8 outer-sum matmul with
bf16 hi/lo split operands. Output: [T->A, C_RGB] fp16 planes DMA'd out,
assembled to f32 on host.
"""

import sys

if "/opt/trn_rl_repo" not in sys.path:
    sys.path.insert(0, "/opt/trn_rl_repo")

import numpy as np
import ml_dtypes

import concourse.bass as bass
import concourse.bacc as bacc
import concourse.mybir as mybir
from concourse.tile import TileContext
from concourse import bass_utils

H = 1024
W = 1024
ROWS = 128
N_CORES = 8
MARGIN = 5.0
ROUND = 8
WMIN = 16
WCAP = 224
F32 = mybir.dt.float32
F16 = mybir.dt.float16
BF16 = mybir.dt.bfloat16
AF = mybir.ActivationFunctionType
OP = mybir.AluOpType
BF = ml_dtypes.bfloat16


# ---------------------------------------------------------------- host plan
def _core_circles(centers, radii, core):
    """Kept circle indices (descending z = front-to-back) + widths/offsets."""
    y0 = ROWS * core
    cy = centers[:, 1].astype(np.float64)
    cx = centers[:, 0].astype(np.float64)
    r = radii.astype(np.float64)
    keep = (cy + r + MARGIN >= y0 + 0.5) & (cy - r - MARGIN <= y0 + ROWS - 0.5)
    idx = np.where(keep)[0][::-1]  # descending index = front-to-back
    dymin = np.maximum(0.0, np.maximum(y0 + 0.5 - cy[idx],
                                       cy[idx] - (y0 + ROWS - 0.5)))
    rm = r[idx] + MARGIN
    halfw = np.sqrt(np.maximum(rm * rm - dymin * dymin, 4.0))
    ws = np.clip(np.ceil(2.0 * halfw / ROUND) * ROUND, WMIN, WCAP).astype(int)
    off = np.clip(np.round(cx[idx] - ws / 2.0), 0, W - ws).astype(int)
    return idx, ws, off


def make_plan(centers, radii, colors):
    """Per-core plan dicts with all compile-time constants."""
    col = colors.astype(np.float64)
    r = radii.astype(np.float64)
    cx = centers[:, 0].astype(np.float64)
    cy = centers[:, 1].astype(np.float64)
    plans = []
    for core in range(N_CORES):
        ids, ws, off = _core_circles(centers, radii, core)
        n = len(ids)
        npairs = (n + 1) // 2
        pairw = []
        for p in range(npairs):
            w0 = int(ws[2 * p])
            w1 = int(ws[2 * p + 1]) if 2 * p + 1 < n else 0
            pairw.append(w0 + w1)
        plans.append({
            "core": core, "ids": ids, "ws": ws.astype(int), "off": off,
            "n": n, "npairs": npairs, "pairw": pairw,
            "alpha": col[ids, 3], "colr": col[ids, 0], "colg": col[ids, 1],
            "colb": col[ids, 2],
            "r": r[ids], "cx": cx[ids], "cy": cy[ids],
        })
    return plans


def _hilo(x):
    hi = x.astype(BF)
    lo = (x - hi.astype(np.float64)).astype(BF)
    return hi, lo


def make_inputs(plans):
    """rhs is group-strided: group g's 4 pairs are packed from DRAM offset
    g*1792 (pair_pos = within-group cumsum), so every core's group-g DMA
    reads the same DRAM range."""
    npairs_max = max(p["npairs"] for p in plans)
    ngroups_max = max((p["n"] + 7) // 8 for p in plans)
    # one DRAM tensor, group-strided: [4*ROWS lhsT chunk | 1792 rhs chunk]
    GSTR = 4 * ROWS + 1792
    ins = []
    for p in plans:
        core = p["core"]
        y0 = ROWS * core
        n = p["n"]
        op = np.zeros((8, ngroups_max * GSTR), BF)
        rows = y0 + np.arange(ROWS, dtype=np.float64) + 0.5
        for k in range(n):
            vk = int(p["ws"][k])
            off = int(p["off"][k])
            ri, cxi, cyi = p["r"][k], p["cx"][k], p["cy"][k]
            j = off + np.arange(vk, dtype=np.float64) + 0.5
            a = ri / 2.0 - (rows - cyi) ** 2 / ri
            b = ri / 2.0 - (j - cxi) ** 2 / ri
            ah, alo = _hilo(a)
            bh, blo = _hilo(b)
            pair, half = divmod(k, 2)
            rb = 4 * half
            g, ip = divmod(pair, 4)
            l0 = g * GSTR + ip * ROWS
            ls = slice(l0, l0 + ROWS)
            op[rb + 0, ls] = ah
            op[rb + 1, ls] = alo
            op[rb + 2, ls] = 1.0
            op[rb + 3, ls] = 1.0
            c0 = g * GSTR + 4 * ROWS + sum(p["pairw"][4 * g:pair]) \
                + (0 if half == 0 else int(p["ws"][2 * pair]))
            rs = slice(c0, c0 + vk)
            op[rb + 0, rs] = 1.0
            op[rb + 1, rs] = 1.0
            op[rb + 2, rs] = bh
            op[rb + 3, rs] = blo
        ins.append({"opnd": op})
    return ins, npairs_max, ngroups_max


# ------------------------------------------------- premult engine balancer
def _assign_premults(ws, npairs, pairw):
    """Greedy per-core assignment of each slot's 3 premult channels to ONE
    engine (ACT/Pool/DVE) — single producer per slot keeps the C-add's
    cross-engine sync to one semaphore. Costs in ns, immediates."""
    n = len(ws)
    dve = act = pool = 0.0
    for k in range(n):
        vk = float(ws[k])
        dve += (58 + vk / 2) / 0.96 + 80          # TT w'
        dve += (58 + vk) / 0.96 + 150             # stt imm
        dve += (58 + 3 * vk / 2) / 0.96 + 170     # 3-plane C add
    for pw in pairw:
        act += (172 + pw) / 1.2 + 60              # sigmoid PSUM->SBUF
    cost = {
        "dve": lambda vk: 3 * ((58 + vk / 4) / 0.96 + 60),
        "act": lambda vk: 3 * ((224 + vk) / 1.2 + 60),
        "pool": lambda vk: 3 * (250 + vk * 1.2 / 1.2),
    }
    busy = {"dve": dve, "act": act, "pool": pool}
    assign = []
    for k in range(n):
        vk = float(ws[k])
        if k >= n - 8:
            e = "dve"  # taper: let ACT/Pool drain so the tail isn't stalled
        else:
            e = min(busy, key=lambda e: busy[e] + cost[e](vk))
        busy[e] += cost[e](vk)
        assign.append((e, e, e))
    return assign


# ------------------------------------------------------------- device build
def build_nc(plans, npairs_max, ngroups_max):
    GSTR = 4 * ROWS + 1792
    nc = bacc.Bacc("TRN2", target_bir_lowering=False, debug=False,
                   num_devices=N_CORES)
    opnd_d = nc.dram_tensor("opnd", [8, ngroups_max * GSTR], BF16,
                            kind="ExternalInput").ap()
    out_d = nc.dram_tensor("out", [ROWS, 4 * W], F16,
                           kind="ExternalOutput").ap()

    with TileContext(nc) as tc:
        T = nc.alloc_sbuf_tensor("T", [ROWS, W], F16).ap()
        CC = nc.alloc_sbuf_tensor("CC", [ROWS, 3 * W], F16).ap()
        AT = nc.alloc_sbuf_tensor("AT", [ROWS, W], F16).ap()
        covr = nc.alloc_sbuf_tensor("covr", [ROWS, 2 * 1792], F16).ap()
        wr = nc.alloc_sbuf_tensor("wr", [ROWS, 8 * WCAP], F16).ap()
        mr = nc.alloc_sbuf_tensor("mr", [ROWS, 8 * 3 * WCAP], F16).ap()
        op_sb = nc.alloc_sbuf_tensor("op_sb", [8, 2 * GSTR], BF16).ap()
        pt0 = nc.alloc_psum_tensor("pt0", [ROWS, 4 * 512], F32).ap()
        pt1 = nc.alloc_psum_tensor("pt1", [ROWS, 4 * 512], F32).ap()
        pts = [pt0, pt1]

        pid = nc.partition_id()
        from concourse.expressions import s_valid_engines
        hint = tc.switch_hint({e: pid for e in s_valid_engines(pid)},
                              N_CORES, label="corearm")

        nc.vector.memset(T, 1.0)
        nc.gpsimd.memset(CC, 0.0)

        CC3 = CC.rearrange("p (c x) -> p c x", x=W)
        mr3 = mr.rearrange("p (s x) -> p s x", x=WCAP)

        # prefetch the first two groups' operands before the dispatch —
        # group-strided DRAM layout makes the ranges identical on all cores
        for g in (0, 1):
            if g < ngroups_max:
                nc.sync.dma_start(op_sb[:, (g % 2) * GSTR:(g % 2 + 1) * GSTR],
                                  opnd_d[:, g * GSTR:(g + 1) * GSTR])

        for core in tc.Switch(pid, N_CORES, hint=hint):
            p = plans[core]
            n = p["n"]
            ws, off = p["ws"], p["off"]
            pairw = p["pairw"]
            assign = _assign_premults(ws, p["npairs"], pairw)
            ngroups = (n + 7) // 8
            pend = []
            for g in range(ngroups):
                k0 = g * 8
                gsize = min(8, n - k0)
                gp = (gsize + 1) // 2
                p0 = k0 // 2
                lh = op_sb[:, (g % 2) * GSTR:]
                rh = op_sb[:, (g % 2) * GSTR + 4 * ROWS:]
                if g >= 2:
                    nc.sync.dma_start(
                        op_sb[:, (g % 2) * GSTR:(g % 2 + 1) * GSTR],
                        opnd_d[:, g * GSTR:(g + 1) * GSTR])
                pt = pts[g % 2]
                cbase = (g % 2) * 1792
                rpos = 0
                pair_pos = []
                for i in range(gp):
                    pw = pairw[p0 + i]
                    pair_pos.append(rpos)
                    nc.tensor.matmul(
                        pt[:, i * 512:i * 512 + pw],
                        lh[:, i * ROWS:(i + 1) * ROWS],
                        rh[:, rpos:rpos + pw],
                        start=True, stop=True)
                    rpos += pw

                rpos = 0
                for j in range(gsize):
                    k = k0 + j
                    vk = int(ws[k])
                    ok = int(off[k])
                    if j % 2 == 0:
                        i = j // 2
                        pw = pairw[p0 + i]
                        nc.scalar.activation(
                            covr[:, cbase + pair_pos[i]:
                                 cbase + pair_pos[i] + pw],
                            pt[:, i * 512:i * 512 + pw], AF.Sigmoid)
                    cov = covr[:, cbase + rpos:cbase + rpos + vk]
                    rpos += vk
                    wv = wr[:, (k % 8) * WCAP:(k % 8) * WCAP + vk]
                    tw = T[:, ok:ok + vk]
                    al = float(p["alpha"][k])
                    acr = float(p["alpha"][k] * p["colr"][k])
                    acg = float(p["alpha"][k] * p["colg"][k])
                    acb = float(p["alpha"][k] * p["colb"][k])
                    # DVE chain: w = cov*T ; T = (-a)*w + T
                    nc.vector.tensor_tensor(wv, cov, tw, OP.mult)
                    nc.vector.scalar_tensor_tensor(tw, wv, -al, tw,
                                                   OP.mult, OP.add)
                    # premults m_ch = (a col_ch) * w, engine per balancer
                    for ch, sc in enumerate((acr, acg, acb)):
                        mb = ((k % 8) * 3 + ch) * WCAP
                        eng = assign[k][ch]
                        if eng == "dve":
                            nc.vector.tensor_scalar(
                                mr[:, mb:mb + vk], wv, sc, 0.0,
                                OP.mult, OP.add)
                        elif eng == "pool":
                            nc.gpsimd.tensor_scalar(
                                mr[:, mb:mb + vk], wv, sc, 0.0,
                                OP.mult, OP.add)
                        else:
                            nc.scalar.activation(
                                mr[:, mb:mb + vk], wv, AF.Copy, scale=sc)
                    # delayed batched C add (2 slots behind)
                    pend.append((k, vk, ok))
                    if len(pend) > 2:
                        kp, vp, op_ = pend.pop(0)
                        qv = CC3[:, :, op_:op_ + vp]
                        mp = mr3[:, (kp % 8) * 3:(kp % 8) * 3 + 3, :vp]
                        nc.vector.tensor_tensor(qv, qv, mp, OP.add)
            for kp, vp, op_ in pend:
                qv = CC3[:, :, op_:op_ + vp]
                mp = mr3[:, (kp % 8) * 3:(kp % 8) * 3 + 3, :vp]
                nc.vector.tensor_tensor(qv, qv, mp, OP.add)

        # A = 1 - T, then plane DMAs
        nc.vector.tensor_scalar(AT, T, -1.0, 1.0, OP.mult, OP.add)
        nc.sync.dma_start(out_d[:, 0:3 * W], CC)
        nc.sync.dma_start(out_d[:, 3 * W:4 * W], AT)

    nc.compile()
    return nc


def kernel(centers, radii, colors):
    centers = np.asarray(centers, np.float32)
    radii = np.asarray(radii, np.float32)
    colors = np.asarray(colors, np.float32)

    plans = make_plan(centers, radii, colors)
    ins, npairs_max, ngroups_max = make_inputs(plans)
    nc = build_nc(plans, npairs_max, ngroups_max)
    res = bass_utils.run_bass_kernel_spmd(nc, ins, list(range(N_CORES)),
                                          trace=False)
    out = np.empty((H, W, 4), np.float32)
    for c in range(N_CORES):
        planes = res.results[c]["out"].astype(np.float32)  # [128, 4*W]
        for ch in range(4):
            out[c * ROWS:(c + 1) * ROWS, :, ch] = planes[:, ch * W:(ch + 1) * W]
    return out
